# revision 1
# baseline (speedup 1.0000x reference)
"""AdaLN attention block (DiT-style) on 8 TRN2 NeuronCores.

Sharding: 8 cores = 4 batches x 2 token-halves, no collectives. Core c handles
batch c//2 and query-token half c%2: layernorm1 and k/v are computed over the
full (permuted) sequence, everything else only for the own 512 query rows.

Device layout is feature-major (activations transposed, [d, n]). X @ W runs
with W column-tiles stationary and X^T moving, producing Y^T directly.
LayerNorm statistics use ones-vector matmuls (partition-axis sums on the PE);
the AdaLN modulate is h = x*A + B with rank-1 A/B built by K=1 outer-product
matmuls into PSUM. Softmax skips max-subtraction (fp32 exp is safe for this
distribution); the denominator is a ones-column appended to the attn@v
stationary operand; normalization is folded in per head via a broadcast
reciprocal.
"""

import numpy as np
from contextlib import ExitStack

import concourse.bass as bass
import concourse.bacc as bacc
import concourse.mybir as mybir
from concourse import tile
from concourse.bass_utils import run_bass_kernel_spmd

P = 128
D = 1024
N = 1024
NQ = 512
H = 16
DH = 64
MLPD = 4096
EPS = 1e-6
NCORES = 8

F32 = mybir.dt.float32
BF16 = mybir.dt.bfloat16
AF = mybir.ActivationFunctionType
ALU = mybir.AluOpType

KT = D // P           # 8 contraction tiles over D
MT = MLPD // P        # 32 tiles over MLP dim


def _r(ap):
    return ap


def build():
    nc = bacc.Bacc("TRN2", target_bir_lowering=False, debug=False,
                   num_devices=NCORES)

    xT = nc.dram_tensor("xT", [D, N], F32, kind="ExternalInput")
    crow = nc.dram_tensor("crow", [1, D], F32, kind="ExternalInput")
    Wq = nc.dram_tensor("Wq", [D, D], BF16, kind="ExternalInput")
    Wkv = nc.dram_tensor("Wkv", [D, 2 * D], BF16, kind="ExternalInput")
    Wo = nc.dram_tensor("Wo", [D, D], BF16, kind="ExternalInput")
    W1 = nc.dram_tensor("W1", [D, MLPD], BF16, kind="ExternalInput")
    W2 = nc.dram_tensor("W2", [MLPD, D], BF16, kind="ExternalInput")
    Wada = nc.dram_tensor("Wada", [D, 6 * D], BF16, kind="ExternalInput")
    bada_r = nc.dram_tensor("bada_r", [1, 6 * D], F32, kind="ExternalInput")
    bq_c = nc.dram_tensor("bq_c", [P, KT], F32, kind="ExternalInput")
    bk_c = nc.dram_tensor("bk_c", [P, KT], F32, kind="ExternalInput")
    bv_c = nc.dram_tensor("bv_c", [P, KT], BF16, kind="ExternalInput")
    bo_r = nc.dram_tensor("bo_r", [1, D], F32, kind="ExternalInput")
    b1_c = nc.dram_tensor("b1_c", [P, MT], F32, kind="ExternalInput")
    b2_c = nc.dram_tensor("b2_c", [P, KT], F32, kind="ExternalInput")
    yT = nc.dram_tensor("yT", [D, NQ], F32, kind="ExternalOutput")

    with tile.TileContext(nc) as tc, ExitStack() as root:
        const = root.enter_context(tc.tile_pool(name="const", bufs=1))
        rootrows = root.enter_context(tc.tile_pool(name="rootrows", bufs=1))

        ones_col = const.tile([P, 1], BF16, name='ones_col')
        nc.vector.memset(ones_col[:], 1.0)
        ones_col_f = const.tile([P, 1], F32, name='ones_col_f')
        nc.vector.memset(ones_col_f[:], 1.0)
        ones_row = const.tile([1, NQ], BF16, name='ones_row')
        nc.vector.memset(ones_row[:], 1.0)
        eps_t = const.tile([1, 1], F32, name='eps_t')
        nc.vector.memset(eps_t[:], EPS)

        bqT = const.tile([P, KT], F32, name='bqT')
        nc.sync.dma_start(bqT[:], bq_c[:])
        bkT = const.tile([P, KT], F32, name='bkT')
        nc.sync.dma_start(bkT[:], bk_c[:])
        bvT = const.tile([P, KT], BF16, name='bvT')
        nc.sync.dma_start(bvT[:], bv_c[:])
        b1T = const.tile([P, MT], F32, name='b1T')
        nc.sync.dma_start(b1T[:], b1_c[:])
        b2T = const.tile([P, KT], F32, name='b2T')
        nc.sync.dma_start(b2T[:], b2_c[:])
        bo_row = const.tile([1, D], F32, name='bo_row')
        nc.sync.dma_start(bo_row[:], bo_r[:])

        bqT_s = const.tile([P, KT], F32, name='bqT_s')
        nc.vector.tensor_scalar_mul(bqT_s[:], bqT[:], DH ** -0.5)

        def cols_from_row(row_ap, dst, psum_pool):
            """[1, n*128] row -> [128, n] column tile via K=1 matmuls."""
            n = dst.shape[-1]
            ps = psum_pool.tile([P, n], F32, tag="colps", name='colps')
            for j in range(n):
                nc.tensor.matmul(ps[:, j:j + 1],
                                 lhsT=_r(row_ap[0:1, j * P:(j + 1) * P]),
                                 rhs=_r(ones_row[0:1, 0:1]),
                                 start=True, stop=True)
            nc.vector.tensor_copy(dst[:], ps[:])
            return dst

        csT = const.tile([P, KT], BF16, name='csT')
        gmsaT = const.tile([P, KT], F32, name='gmsaT')
        gmlpT = const.tile([P, KT], F32, name='gmlpT')

        # persistent activation arrays (distinct tag per tile, 1 buf each)
        op_cm = tc.tile_pool(name="op", bufs=1, side='left')
        op_ = op_cm.__enter__()
        outT = [op_.tile([P, NQ], BF16, tag=f"o{k}", name=f"o{k}")
                for k in range(KT)]
        hT_cm = tc.tile_pool(name="hTp", bufs=1, side='left')
        hTp = hT_cm.__enter__()
        hT = [hTp.tile([P, N], BF16, tag=f"h{k}", name=f"h{k}")
              for k in range(KT)]

        S2_row = rootrows.tile([1, D], BF16, name='S2_row')
        sh2_row_t = rootrows.tile([1, D], BF16, name='sh2_row_t')

        # ---------------- phase 0+1: mod vector & ln1 ----------------
        with ExitStack() as ph:
            rows = ph.enter_context(tc.tile_pool(name="p0rows", bufs=1))
            badpool = ph.enter_context(tc.tile_pool(name="p0bad", bufs=2))
            xpool = ph.enter_context(tc.tile_pool(name="p0x", bufs=1))
            sqpool = ph.enter_context(tc.tile_pool(name="p0sq", bufs=4))
            wpool = ph.enter_context(tc.tile_pool(name="p0w", bufs=26))

            xt = [xpool.tile([P, N], F32, tag=f"x{k}", name=f"x{k}")
                  for k in range(KT)]
            for k in range(KT):
                eng = nc.sync if k % 2 == 0 else nc.gpsimd
                eng.dma_start(xt[k][:, 0:NQ], xT[k * P:(k + 1) * P, 0:NQ])
                eng2 = nc.gpsimd if k % 2 == 0 else nc.sync
                eng2.dma_start(xt[k][:, NQ:N], xT[k * P:(k + 1) * P, NQ:N])

            with ExitStack() as sec:
                pscol = sec.enter_context(
                    tc.tile_pool(name="pscol", bufs=1, space="PSUM"))
                psmod = sec.enter_context(
                    tc.tile_pool(name="psmod", bufs=3, space="PSUM"))
                psstat = sec.enter_context(
                    tc.tile_pool(name="psstat", bufs=2, space="PSUM"))

                # silu(c) and its column layout
                c_sb = rows.tile([1, D], F32, name='c_sb')
                nc.sync.dma_start(c_sb[:], crow[:])
                cs_row = rows.tile([1, D], BF16, name='cs_row')
                nc.scalar.activation(cs_row[:], c_sb[:], AF.Silu)
                cols_from_row(cs_row, csT, pscol)

                # mod = silu(c) @ Wada + bada   [1, 6144]
                mod_row = rows.tile([1, 6 * D], BF16, name='mod_row')
                for g in range(12):
                    wch = [wpool.tile([P, NQ], BF16, tag="wada",
                                      name='wada') for _ in range(KT)]
                    for k in range(KT):
                        eng = nc.sync if k % 2 == 0 else nc.gpsimd
                        eng.dma_start(
                            wch[k][:], Wada[k * P:(k + 1) * P,
                                            g * NQ:(g + 1) * NQ])
                    mp = psmod.tile([1, NQ], F32, tag="modps", name='modps')
                    for k in range(KT):
                        nc.tensor.matmul(mp[:], lhsT=_r(csT[:, k:k + 1]),
                                         rhs=_r(wch[k][:]),
                                         start=(k == 0), stop=(k == KT - 1))
                    bad = badpool.tile([1, NQ], F32, tag="bad", name='bad')
                    nc.sync.dma_start(bad[:], bada_r[0:1,
                                                     g * NQ:(g + 1) * NQ])
                    nc.vector.tensor_add(
                        mod_row[0:1, g * NQ:(g + 1) * NQ], mp[:], bad[:])

                # ln1 stats: per 512-chunk, sum and sumsq over d
                mu_row = rows.tile([1, N], F32, name='mu_row')
                ex2_row = rows.tile([1, N], F32, name='ex2_row')
                for ch in range(2):
                    sl = slice(ch * NQ, (ch + 1) * NQ)
                    ss = psstat.tile([1, NQ], F32, tag="st_s", name='st_s')
                    sq_ps = psstat.tile([1, NQ], F32, tag="st_q",
                                        name='st_q')
                    for k in range(KT):
                        xb = sqpool.tile([P, NQ], BF16, tag="xb", name='xb')
                        nc.vector.tensor_copy(xb[:], xt[k][:, sl])
                        sq = sqpool.tile([P, NQ], BF16, tag="xsq",
                                         name='xsq')
                        nc.vector.tensor_mul(sq[:], xt[k][:, sl],
                                             xt[k][:, sl])
                        nc.tensor.matmul(ss[:], lhsT=_r(ones_col[:]),
                                         rhs=_r(xb[:]),
                                         start=(k == 0), stop=(k == KT - 1))
                        nc.tensor.matmul(sq_ps[:], lhsT=_r(ones_col[:]),
                                         rhs=_r(sq[:]),
                                         start=(k == 0), stop=(k == KT - 1))
                    nc.vector.tensor_scalar_mul(mu_row[0:1, sl], ss[:],
                                                1.0 / D)
                    nc.vector.tensor_scalar_mul(ex2_row[0:1, sl], sq_ps[:],
                                                1.0 / D)

                var_row = rows.tile([1, N], F32, name='var_row')
                nc.vector.tensor_mul(var_row[:], mu_row[:], mu_row[:])
                nc.vector.tensor_sub(var_row[:], ex2_row[:], var_row[:])
                sd_row = rows.tile([1, N], F32, name='sd_row')
                nc.scalar.activation(sd_row[:], var_row[:], AF.Sqrt,
                                     bias=eps_t[:])
                a_row = rows.tile([1, N], BF16, name='a_row')
                with nc.allow_low_precision(reason="istd bf16 for matmul"):
                    nc.vector.reciprocal(a_row[:], sd_row[:])
                b_row = rows.tile([1, N], BF16, name='b_row')
                nc.vector.tensor_mul(b_row[:], mu_row[:], a_row[:])
                nc.vector.tensor_scalar_mul(b_row[:], b_row[:], -1.0)

                # modulation rows / columns
                S1_row = rows.tile([1, D], BF16, name='S1_row')
                nc.vector.tensor_scalar_add(S1_row[:],
                                            mod_row[0:1, D:2 * D], 1.0)
                nc.vector.tensor_scalar_add(S2_row[:],
                                            mod_row[0:1, 4 * D:5 * D], 1.0)
                nc.vector.tensor_copy(sh2_row_t[:],
                                      mod_row[0:1, 3 * D:4 * D])
                cols_from_row(mod_row[0:1, 2 * D:3 * D], gmsaT, pscol)
                cols_from_row(mod_row[0:1, 5 * D:6 * D], gmlpT, pscol)
                sh1_row = mod_row[0:1, 0:D]

            # h = x*A + B  (A = S1 (x) a, B = S1 (x) b + sh1 (x) 1)
            with ExitStack() as sec:
                psab = sec.enter_context(
                    tc.tile_pool(name="psab", bufs=2, space="PSUM"))
                for k in range(KT):
                    for ch in range(2):
                        sl = slice(ch * NQ, (ch + 1) * NQ)
                        pa = psab.tile([P, NQ], F32, tag="pA", name='pA')
                        pb = psab.tile([P, NQ], F32, tag="pB", name='pB')
                        nc.tensor.matmul(
                            pa[:], lhsT=_r(S1_row[0:1, k * P:(k + 1) * P]),
                            rhs=_r(a_row[0:1, sl]), start=True, stop=True)
                        nc.tensor.matmul(
                            pb[:], lhsT=_r(S1_row[0:1, k * P:(k + 1) * P]),
                            rhs=_r(b_row[0:1, sl]), start=True, stop=False)
                        nc.tensor.matmul(
                            pb[:], lhsT=_r(sh1_row[0:1, k * P:(k + 1) * P]),
                            rhs=_r(ones_row[:]), start=False, stop=True)
                        nc.vector.tensor_mul(hT[k][:, sl], xt[k][:, sl],
                                             pa[:])
                        nc.vector.tensor_add(hT[k][:, sl], hT[k][:, sl],
                                             pb[:])

        # ---------------- phase 2: q, k, v projections ----------------
        qkv_cm = tc.tile_pool(name="qkvp", bufs=1, side='right')
        qkvp = qkv_cm.__enter__()
        qTt = [qkvp.tile([P, NQ], BF16, tag=f"q{k}", name=f"q{k}")
               for k in range(KT)]
        kTt = [qkvp.tile([P, N], BF16, tag=f"k{k}", name=f"k{k}")
               for k in range(KT)]
        vRt = [qkvp.tile([P, H * (DH + 1)], BF16, tag=f"v{k}", name=f"v{k}")
               for k in range(KT)]

        wkv_cm = tc.tile_pool(name="wkvp", bufs=1, side='right')
        wkvp = wkv_cm.__enter__()
        wkc = {}   # (g) -> k-part chunks; ('v', vg) -> v-part chunks
        for g in range(2):
            wkc[g] = [wkvp.tile([P, NQ], BF16, tag=f"kg{g}_{k}",
                                name=f"kg{g}_{k}") for k in range(KT)]
            for k in range(KT):
                eng = nc.sync if k % 2 == 0 else nc.gpsimd
                eng.dma_start(wkc[g][k][:],
                              Wkv[k * P:(k + 1) * P, g * NQ:(g + 1) * NQ])
        for vg in range(2):
            wkc['v', vg] = [wkvp.tile([P, NQ], BF16, tag=f"vg{vg}_{k}",
                                      name=f"vg{vg}_{k}")
                            for k in range(KT)]
            for k in range(KT):
                eng = nc.sync if k % 2 == 0 else nc.gpsimd
                eng.dma_start(wkc['v', vg][k][:],
                              Wkv[k * P:(k + 1) * P,
                                  D + vg * NQ:D + (vg + 1) * NQ])

        prj_cm = tc.tile_pool(name="prjps", bufs=1, space="PSUM",
                              side='right')
        prjps = prj_cm.__enter__()

        def emit_kT(t, ch):
            g, dot = t // 4, t % 4
            sl = slice(ch * NQ, (ch + 1) * NQ)
            p = prjps.tile([P, NQ], F32, tag="prj", name='prj')
            for k in range(KT):
                nc.tensor.matmul(
                    p[:], lhsT=_r(wkc[g][k][:, dot * P:(dot + 1) * P]),
                    rhs=_r(hT[k][:, sl]),
                    start=(k == 0), stop=(k == KT - 1))
            nc.scalar.activation(kTt[t][:, sl], p[:], AF.Identity,
                                 bias=bkT[:, t:t + 1])

        def emit_v(vg, nt):
            p = prjps.tile([P, NQ], F32, tag="prj", name='prj')
            for k in range(KT):
                nc.tensor.matmul(
                    p[:], lhsT=_r(hT[k][:, nt * P:(nt + 1) * P]),
                    rhs=_r(wkc['v', vg][k][:]),
                    start=(k == 0), stop=(k == KT - 1))
            vv = vRt[nt].rearrange("p (h w) -> p h w", w=DH + 1)
            pv = p.rearrange("p (h w) -> p h w", w=DH)
            nc.vector.tensor_copy(vv[:, vg * 8:(vg + 1) * 8, 0:DH], pv[:])

        with ExitStack() as ph:
            wpool = ph.enter_context(tc.tile_pool(name="p2w", bufs=26))
            ps = ph.enter_context(
                tc.tile_pool(name="p2ps", bufs=6, space="PSUM"))

            for nt in range(KT):
                vv = vRt[nt].rearrange("p (h w) -> p h w", w=DH + 1)
                nc.vector.memset(vv[:, :, DH:DH + 1], 1.0)

            def stationary_group(wdram, col0, movs, evict, tagp):
                wch = [wpool.tile([P, NQ], BF16, tag=tagp, name=tagp)
                       for _ in range(KT)]
                for k in range(KT):
                    eng = nc.sync if k % 2 == 0 else nc.gpsimd
                    eng.dma_start(
                        wch[k][:], wdram[k * P:(k + 1) * P, col0:col0 + NQ])
                for dot in range(4):
                    p = ps.tile([P, NQ], F32, tag="prj", name='prj')
                    for k in range(KT):
                        nc.tensor.matmul(
                            p[:], lhsT=_r(wch[k][:, dot * P:(dot + 1) * P]),
                            rhs=movs[k], start=(k == 0), stop=(k == KT - 1))
                    evict(dot, p)

            # q^T (own rows), scaled by 1/sqrt(DH)
            for g in range(2):
                def ev_q(dot, p, g=g):
                    t = 4 * g + dot
                    nc.scalar.activation(qTt[t][:], p[:], AF.Identity,
                                         bias=bqT_s[:, t:t + 1],
                                         scale=DH ** -0.5)
                stationary_group(Wq, g * NQ,
                                 [_r(hT[k][:, 0:NQ]) for k in range(KT)],
                                 ev_q, "wst")

            # k^T tiles 0-1 and v-group 0 now; the rest is emitted inside
            # the attention loop as just-in-time full-array work that keeps
            # the PE clock warm
            for t in range(2):
                emit_kT(t, 0)
                emit_kT(t, 1)
            for nt in range(KT):
                emit_v(0, nt)

        # ---------------- phase 3: attention ----------------

        with ExitStack() as ph:
            epool = ph.enter_context(tc.tile_pool(name="p3e", bufs=52))
            spool = ph.enter_context(tc.tile_pool(name="p3s", bufs=4))
            ps_sim = ph.enter_context(
                tc.tile_pool(name="ps_sim", bufs=3, space="PSUM"))
            ps_bc = ph.enter_context(
                tc.tile_pool(name="ps_bc", bufs=1, space="PSUM"))
            ps_o = ph.enter_context(
                tc.tile_pool(name="ps_o", bufs=3, space="PSUM"))

            for hp in range(H // 2):
                pt = hp
                if 0 < hp < 7:
                    emit_kT(hp + 1, 0)
                    emit_kT(hp + 1, 1)
                if hp < 4:
                    emit_v(1, 2 * hp)
                    emit_v(1, 2 * hp + 1)
                et = {0: [], 1: []}
                for kt in range(KT):
                    pp = {}
                    for hi in range(2):
                        hh = hi * DH
                        p = ps_sim.tile([P, NQ], F32, tag="sim",
                                        name='sim')
                        nc.tensor.matmul(
                            p[:],
                            lhsT=_r(kTt[pt][hh:hh + DH,
                                            kt * P:(kt + 1) * P]),
                            rhs=_r(qTt[pt][hh:hh + DH, :]),
                            start=True, stop=True)
                        pp[hi] = p
                    for hi in range(2):
                        e = epool.tile([P, NQ], BF16, tag="e", name='e')
                        nc.scalar.activation(e[:], pp[hi][:], AF.Exp)
                        et[hi].append(e)
                pos = {}
                for hi in range(2):
                    pos[hi] = ps_o.tile([DH + 1, NQ], F32, tag="ov",
                                        name='ov')
                for kt in range(KT):
                    for hi in range(2):
                        h = 2 * hp + hi
                        nc.tensor.matmul(
                            pos[hi][:],
                            lhsT=_r(vRt[kt][:, h * (DH + 1):
                                            (h + 1) * (DH + 1)]),
                            rhs=_r(et[hi][kt][:]),
                            start=(kt == 0), stop=(kt == KT - 1))
                for hi in range(2):
                    hh = hi * DH
                    po = pos[hi]
                    inv_s = spool.tile([1, NQ], BF16, tag="invs",
                                       name='invs')
                    with nc.allow_low_precision(reason="softmax denom"):
                        nc.vector.reciprocal(inv_s[:], po[DH:DH + 1, :])
                    pb = ps_bc.tile([DH, NQ], F32, tag="bc", name='bc')
                    nc.tensor.matmul(pb[:], lhsT=_r(ones_row[0:1, 0:DH]),
                                     rhs=_r(inv_s[:]), start=True,
                                     stop=True)
                    binv = spool.tile([DH, NQ], F32, tag="binv",
                                      name='binv')
                    nc.vector.tensor_copy(binv[:], pb[:])
                    nc.vector.tensor_mul(outT[pt][hh:hh + DH, :],
                                         po[0:DH, :], binv[:])

        prj_cm.__exit__(None, None, None)
        wkv_cm.__exit__(None, None, None)
        qkv_cm.__exit__(None, None, None)
        hT_cm.__exit__(None, None, None)

        # ---------------- phase 4: Wo + residual + ln2 ----------------
        x1p = root.enter_context(tc.tile_pool(name="x1p", bufs=1, side='right'))
        x1t = [x1p.tile([P, NQ], F32, tag=f"x1{k}", name=f"x1{k}")
               for k in range(KT)]
        h2p = root.enter_context(tc.tile_pool(name="h2p", bufs=1, side='right'))
        h2t = [h2p.tile([P, NQ], BF16, tag=f"h2{k}", name=f"h2{k}")
               for k in range(KT)]

        with ExitStack() as ph:
            rows4 = ph.enter_context(tc.tile_pool(name="p4rows", bufs=1))
            wpool = ph.enter_context(tc.tile_pool(name="p4w", bufs=10))
            xpool = ph.enter_context(tc.tile_pool(name="p4x", bufs=1))
            tpool = ph.enter_context(tc.tile_pool(name="p4t", bufs=3))

            xo = [xpool.tile([P, NQ], F32, tag=f"xo{k}", name=f"xo{k}")
                  for k in range(KT)]
            for k in range(KT):
                eng = nc.sync if k % 2 == 0 else nc.gpsimd
                eng.dma_start(xo[k][:], xT[k * P:(k + 1) * P, 0:NQ])

            bop_row = rows4.tile([1, D], BF16, name='bop_row')
            boT = const.tile([P, KT], F32, name='boT')

            with ExitStack() as sec:
                psv = sec.enter_context(
                    tc.tile_pool(name="psv", bufs=2, space="PSUM"))
                pscol2 = sec.enter_context(
                    tc.tile_pool(name="pscol2", bufs=1, space="PSUM"))
                psy = sec.enter_context(
                    tc.tile_pool(name="psy", bufs=2, space="PSUM"))

                for g in range(2):
                    wch = [wpool.tile([P, NQ], BF16, tag="wo", name='wo')
                           for _ in range(KT)]
                    for k in range(KT):
                        eng = nc.sync if k % 2 == 0 else nc.gpsimd
                        eng.dma_start(
                            wch[k][:],
                            Wo[k * P:(k + 1) * P, g * NQ:(g + 1) * NQ])
                    mp = psv.tile([1, NQ], F32, tag="bvps", name='bvps')
                    for k in range(KT):
                        nc.tensor.matmul(mp[:], lhsT=_r(bvT[:, k:k + 1]),
                                         rhs=_r(wch[k][:]),
                                         start=(k == 0), stop=(k == KT - 1))
                    nc.vector.tensor_add(
                        bop_row[0:1, g * NQ:(g + 1) * NQ], mp[:],
                        bo_row[0:1, g * NQ:(g + 1) * NQ])
                    cols_from_row(bop_row[0:1, g * NQ:(g + 1) * NQ],
                                  boT[:, g * 4:(g + 1) * 4], pscol2)
                    for dot in range(4):
                        t = 4 * g + dot
                        p = psy.tile([P, NQ], F32, tag="y1", name='y1')
                        for k in range(KT):
                            nc.tensor.matmul(
                                p[:],
                                lhsT=_r(wch[k][:, dot * P:(dot + 1) * P]),
                                rhs=_r(outT[k][:]),
                                start=(k == 0), stop=(k == KT - 1))
                        tmp = tpool.tile([P, NQ], F32, tag="y1s",
                                         name='y1s')
                        nc.vector.tensor_scalar(tmp[:], p[:],
                                                boT[:, t:t + 1],
                                                gmsaT[:, t:t + 1],
                                                ALU.add, ALU.mult)
                        nc.vector.tensor_add(x1t[t][:], xo[t][:], tmp[:])

            with ExitStack() as sec:
                psstat2 = sec.enter_context(
                    tc.tile_pool(name="psstat2", bufs=1, space="PSUM"))
                psab2 = sec.enter_context(
                    tc.tile_pool(name="psab2", bufs=2, space="PSUM"))

                ss = psstat2.tile([1, NQ], F32, tag="st2s", name='st2s')
                sq_ps = psstat2.tile([1, NQ], F32, tag="st2q", name='st2q')
                for k in range(KT):
                    sq = tpool.tile([P, NQ], BF16, tag="x1sq", name='x1sq')
                    nc.vector.tensor_mul(sq[:], x1t[k][:], x1t[k][:])
                    nc.tensor.matmul(ss[:], lhsT=_r(ones_col_f[:]),
                                     rhs=_r(x1t[k][:]),
                                     start=(k == 0), stop=(k == KT - 1))
                    nc.tensor.matmul(sq_ps[:], lhsT=_r(ones_col[:]),
                                     rhs=_r(sq[:]),
                                     start=(k == 0), stop=(k == KT - 1))
                mu2 = rows4.tile([1, NQ], F32, name='mu2')
                ex22 = rows4.tile([1, NQ], F32, name='ex22')
                nc.vector.tensor_scalar_mul(mu2[:], ss[:], 1.0 / D)
                nc.vector.tensor_scalar_mul(ex22[:], sq_ps[:], 1.0 / D)
                var2 = rows4.tile([1, NQ], F32, name='var2')
                nc.vector.tensor_mul(var2[:], mu2[:], mu2[:])
                nc.vector.tensor_sub(var2[:], ex22[:], var2[:])
                sd2 = rows4.tile([1, NQ], F32, name='sd2')
                nc.scalar.activation(sd2[:], var2[:], AF.Sqrt,
                                     bias=eps_t[:])
                a2 = rows4.tile([1, NQ], BF16, name='a2')
                with nc.allow_low_precision(reason="istd bf16 for matmul"):
                    nc.vector.reciprocal(a2[:], sd2[:])
                b2r = rows4.tile([1, NQ], BF16, name='b2r')
                nc.vector.tensor_mul(b2r[:], mu2[:], a2[:])
                nc.vector.tensor_scalar_mul(b2r[:], b2r[:], -1.0)

                for k in range(KT):
                    pa = psab2.tile([P, NQ], F32, tag="pA2", name='pA2')
                    pb = psab2.tile([P, NQ], F32, tag="pB2", name='pB2')
                    nc.tensor.matmul(
                        pa[:], lhsT=_r(S2_row[0:1, k * P:(k + 1) * P]),
                        rhs=_r(a2[:]), start=True, stop=True)
                    nc.tensor.matmul(
                        pb[:], lhsT=_r(S2_row[0:1, k * P:(k + 1) * P]),
                        rhs=_r(b2r[:]), start=True, stop=False)
                    nc.tensor.matmul(
                        pb[:], lhsT=_r(sh2_row_t[0:1, k * P:(k + 1) * P]),
                        rhs=_r(ones_row[:]), start=False, stop=True)
                    nc.vector.tensor_mul(h2t[k][:], x1t[k][:], pa[:])
                    nc.vector.tensor_add(h2t[k][:], h2t[k][:], pb[:])

        op_cm.__exit__(None, None, None)

        # ---------------- phase 5: MLP ----------------
        with ExitStack() as ph:
            gp = ph.enter_context(tc.tile_pool(name="gp", bufs=1))
            gTt = [gp.tile([P, NQ], BF16, tag=f"g{m}", name=f"g{m}")
                   for m in range(MT)]
            wpool = ph.enter_context(tc.tile_pool(name="p5w", bufs=24))
            w2pool = ph.enter_context(tc.tile_pool(name="p5w2", bufs=16))
            opool = ph.enter_context(tc.tile_pool(name="p5o", bufs=3))
            ps1 = ph.enter_context(
                tc.tile_pool(name="ps1", bufs=4, space="PSUM"))
            ps2 = ph.enter_context(
                tc.tile_pool(name="ps2", bufs=1, space="PSUM"))

            for g in range(MLPD // NQ):   # 8 column groups
                wch = [wpool.tile([P, NQ], BF16, tag="w1", name='w1')
                       for _ in range(KT)]
                for k in range(KT):
                    eng = nc.sync if k % 2 == 0 else nc.gpsimd
                    eng.dma_start(
                        wch[k][:], W1[k * P:(k + 1) * P,
                                      g * NQ:(g + 1) * NQ])
                for dot in range(4):
                    m = 4 * g + dot
                    p = ps1.tile([P, NQ], F32, tag="m1", name='m1')
                    for k in range(KT):
                        nc.tensor.matmul(
                            p[:], lhsT=_r(wch[k][:, dot * P:(dot + 1) * P]),
                            rhs=_r(h2t[k][:]),
                            start=(k == 0), stop=(k == KT - 1))
                    nc.scalar.activation(gTt[m][:], p[:], AF.Gelu_apprx_tanh,
                                         bias=b1T[:, m:m + 1])

            for half in range(2):
                pacc = [ps2.tile([P, NQ], F32, tag=f"acc{d}",
                                 name=f"acc{d}") for d in range(4)]
                for mk in range(MT):
                    w2c = w2pool.tile([P, NQ], BF16, tag="w2", name='w2')
                    eng = nc.sync if mk % 2 == 0 else nc.gpsimd
                    eng.dma_start(
                        w2c[:], W2[mk * P:(mk + 1) * P,
                                   half * NQ:(half + 1) * NQ])
                    for d in range(4):
                        nc.tensor.matmul(
                            pacc[d][:],
                            lhsT=_r(w2c[:, d * P:(d + 1) * P]),
                            rhs=_r(gTt[mk][:]),
                            start=(mk == 0), stop=(mk == MT - 1))
                for d in range(4):
                    t = half * 4 + d
                    tmp = opool.tile([P, NQ], F32, tag="m2s", name='m2s')
                    nc.vector.tensor_scalar(tmp[:], pacc[d][:],
                                            b2T[:, t:t + 1],
                                            gmlpT[:, t:t + 1],
                                            ALU.add, ALU.mult)
                    yt = opool.tile([P, NQ], F32, tag="yout", name='yout')
                    nc.vector.tensor_add(yt[:], x1t[t][:], tmp[:])
                    eng = nc.sync if t % 2 == 0 else nc.gpsimd
                    eng.dma_start(yT[t * P:(t + 1) * P, :], yt[:])

    nc.compile()
    return nc


_NC = None


def _get_nc():
    global _NC
    if _NC is None:
        _NC = build()
    return _NC


def _prep_inputs(x, c, Wq, bq, Wkv, bkv, Wo, bo, W1, b1, W2, b2, Wada, bada):
    import ml_dtypes
    f = np.float32
    bf = ml_dtypes.bfloat16
    col = lambda v, n: np.ascontiguousarray(
        np.asarray(v, f).reshape(n, P).T)
    shared = {
        "Wq": np.asarray(Wq, f).astype(bf), "Wkv": np.asarray(Wkv, f).astype(bf),
        "Wo": np.asarray(Wo, f).astype(bf), "W1": np.asarray(W1, f).astype(bf),
        "W2": np.asarray(W2, f).astype(bf), "Wada": np.asarray(Wada, f).astype(bf),
        "bada_r": np.asarray(bada, f).reshape(1, -1),
        "bq_c": col(bq, KT), "bk_c": col(np.asarray(bkv, f)[:D], KT),
        "bv_c": col(np.asarray(bkv, f)[D:], KT).astype(bf),
        "bo_r": np.asarray(bo, f).reshape(1, -1),
        "b1_c": col(b1, MT), "b2_c": col(b2, KT),
    }
    in_maps = []
    for core in range(NCORES):
        b, half = core // 2, core % 2
        xb = np.asarray(x[b], f)
        perm = np.concatenate(
            [xb[half * NQ:(half + 1) * NQ],
             xb[(1 - half) * NQ:(2 - half) * NQ]], axis=0)
        m = dict(shared)
        m["xT"] = np.ascontiguousarray(perm.T)
        m["crow"] = np.asarray(c[b:b + 1], f)
        in_maps.append(m)
    return in_maps


def _run(inputs, trace=False):
    nc = _get_nc()
    in_maps = _prep_inputs(**inputs)
    res = run_bass_kernel_spmd(nc, in_maps, core_ids=list(range(NCORES)),
                               trace=trace)
    B = 4
    y = np.empty((B, N, D), np.float32)
    for core in range(NCORES):
        b, half = core // 2, core % 2
        y[b, half * NQ:(half + 1) * NQ, :] = res.results[core]["yT"].T
    return y, res


def kernel(**inputs):
    y, _ = _run(inputs, trace=False)
    return y



# revision 17
# speedup vs baseline: 1.3714x; 1.3714x over previous
"""AdaLN attention block (DiT-style) on 8 TRN2 NeuronCores.

Sharding: 8 cores = 4 batches x 2 token-halves, no collectives. Core c handles
batch c//2 and query-token half c%2: layernorm1 and k/v are computed over the
full (permuted) sequence, everything else only for the own 512 query rows.

Device layout is feature-major (activations transposed, [d, n]). X @ W runs
with W column-tiles stationary and X^T moving, producing Y^T directly.
LayerNorm statistics use ones-vector matmuls (partition-axis sums on the PE);
the AdaLN modulate is h = x*A + B with rank-1 A/B built by K=1 outer-product
matmuls into PSUM. Softmax skips max-subtraction (fp32 exp is safe for this
distribution); the denominator is a ones-column appended to the attn@v
stationary operand; normalization is folded in per head via a broadcast
reciprocal.
"""

import numpy as np
from contextlib import ExitStack

import concourse.bass as bass
import concourse.bacc as bacc
import concourse.mybir as mybir
from concourse import tile
from concourse.bass_utils import run_bass_kernel_spmd

P = 128
D = 1024
N = 1024
NQ = 512
H = 16
DH = 64
MLPD = 4096
EPS = 1e-6
NCORES = 8

F32 = mybir.dt.float32
BF16 = mybir.dt.bfloat16
AF = mybir.ActivationFunctionType
ALU = mybir.AluOpType

KT = D // P           # 8 contraction tiles over D
MT = MLPD // P        # 32 tiles over MLP dim


def _r(ap):
    return ap


def build():
    nc = bacc.Bacc("TRN2", target_bir_lowering=False, debug=False,
                   num_devices=NCORES)

    xT = nc.dram_tensor("xT", [D, N], BF16, kind="ExternalInput")
    crow = nc.dram_tensor("crow", [1, D], F32, kind="ExternalInput")
    Wq = nc.dram_tensor("Wq", [D, D], BF16, kind="ExternalInput")
    Wkv = nc.dram_tensor("Wkv", [D, 2 * D], BF16, kind="ExternalInput")
    Wo = nc.dram_tensor("Wo", [D, D], BF16, kind="ExternalInput")
    W1 = nc.dram_tensor("W1", [D, MLPD], BF16, kind="ExternalInput")
    W2 = nc.dram_tensor("W2", [MLPD, D], BF16, kind="ExternalInput")
    Wada = nc.dram_tensor("Wada", [D, 6 * D], BF16, kind="ExternalInput")
    bada_r = nc.dram_tensor("bada_r", [1, 6 * D], F32, kind="ExternalInput")
    bq_c = nc.dram_tensor("bq_c", [P, KT], F32, kind="ExternalInput")
    bk_c = nc.dram_tensor("bk_c", [P, KT], F32, kind="ExternalInput")
    bv_c = nc.dram_tensor("bv_c", [P, KT], BF16, kind="ExternalInput")
    bo_r = nc.dram_tensor("bo_r", [1, D], F32, kind="ExternalInput")
    b1_c = nc.dram_tensor("b1_c", [P, MT], F32, kind="ExternalInput")
    b2_c = nc.dram_tensor("b2_c", [P, KT], F32, kind="ExternalInput")
    yT = nc.dram_tensor("yT", [D, NQ], BF16, kind="ExternalOutput")

    with tile.TileContext(nc) as tc, ExitStack() as root:
        const = root.enter_context(tc.tile_pool(name="const", bufs=1))
        rootrows = root.enter_context(tc.tile_pool(name="rootrows", bufs=1))

        ones_col = const.tile([P, 1], BF16, name='ones_col')
        nc.vector.memset(ones_col[:], 1.0)
        ones_col_f = const.tile([P, 1], F32, name='ones_col_f')
        nc.vector.memset(ones_col_f[:], 1.0)
        ones_row = const.tile([1, NQ], BF16, name='ones_row')
        nc.vector.memset(ones_row[:], 1.0)
        eps_t = const.tile([1, 1], F32, name='eps_t')
        nc.vector.memset(eps_t[:], EPS)

        bqT = const.tile([P, KT], F32, name='bqT')
        nc.sync.dma_start(bqT[:], bq_c[:])
        bkT = const.tile([P, KT], F32, name='bkT')
        nc.sync.dma_start(bkT[:], bk_c[:])
        bvT = const.tile([P, KT], BF16, name='bvT')
        nc.sync.dma_start(bvT[:], bv_c[:])
        b1T = const.tile([P, MT], F32, name='b1T')
        nc.sync.dma_start(b1T[:], b1_c[:])
        b2T = const.tile([P, KT], F32, name='b2T')
        nc.sync.dma_start(b2T[:], b2_c[:])
        bo_row = const.tile([1, D], F32, name='bo_row')
        nc.sync.dma_start(bo_row[:], bo_r[:])

        bqT_s = const.tile([P, KT], F32, name='bqT_s')
        nc.vector.tensor_scalar_mul(bqT_s[:], bqT[:], DH ** -0.5)

        def cols_from_row(row_ap, dst, psum_pool):
            """[1, n*128] row -> [128, n] column tile via K=1 matmuls."""
            n = dst.shape[-1]
            ps = psum_pool.tile([P, n], F32, tag="colps", name='colps')
            for j in range(n):
                nc.tensor.matmul(ps[:, j:j + 1],
                                 lhsT=_r(row_ap[0:1, j * P:(j + 1) * P]),
                                 rhs=_r(ones_row[0:1, 0:1]),
                                 start=True, stop=True)
            nc.vector.tensor_copy(dst[:], ps[:])
            return dst

        csT = const.tile([P, KT], BF16, name='csT')
        gmsaT = const.tile([P, KT], F32, name='gmsaT')
        gmlpT = const.tile([P, KT], F32, name='gmlpT')

        # persistent activation arrays (distinct tag per tile, 1 buf each)
        op_cm = tc.tile_pool(name="op", bufs=1, side='left')
        op_ = op_cm.__enter__()
        outT = [op_.tile([P, NQ], BF16, tag=f"o{k}", name=f"o{k}")
                for k in range(KT)]
        hT_cm = tc.tile_pool(name="hTp", bufs=1, side='left')
        hTp = hT_cm.__enter__()
        hT = [hTp.tile([P, N], BF16, tag=f"h{k}", name=f"h{k}")
              for k in range(KT)]

        S2_row = rootrows.tile([1, D], BF16, name='S2_row')
        sh2_row_t = rootrows.tile([1, D], BF16, name='sh2_row_t')

        # ---------------- phase 0+1: mod vector & ln1 ----------------
        with ExitStack() as ph:
            rows = ph.enter_context(tc.tile_pool(name="p0rows", bufs=1))
            xpool = ph.enter_context(tc.tile_pool(name="p0x", bufs=1))
            sqpool = ph.enter_context(tc.tile_pool(name="p0sq", bufs=4))
            wpool = ph.enter_context(tc.tile_pool(name="p0w", bufs=26))

            # c + bada first, on the scalar HWDGE queue so they are not
            # stuck behind the bulk x/Wada transfers
            c_sb = rows.tile([1, D], F32, name='c_sb')
            nc.scalar.dma_start(c_sb[:], crow[:])
            bad_full = rows.tile([1, 6 * D], F32, name='bad_full')
            nc.scalar.dma_start(bad_full[:], bada_r[:])

            xt = [xpool.tile([P, N], BF16, tag=f"x{k}", name=f"x{k}")
                  for k in range(KT)]
            for k in range(KT):
                eng = nc.sync if k % 2 == 0 else nc.gpsimd
                eng.dma_start(xt[k][:], xT[k * P:(k + 1) * P, :])

            with ExitStack() as sec:
                pscol = sec.enter_context(
                    tc.tile_pool(name="pscol", bufs=1, space="PSUM"))
                psmod = sec.enter_context(
                    tc.tile_pool(name="psmod", bufs=3, space="PSUM"))
                psstat = sec.enter_context(
                    tc.tile_pool(name="psstat", bufs=2, space="PSUM"))

                # silu(c) and its column layout
                cs_row = rows.tile([1, D], BF16, name='cs_row')
                nc.scalar.activation(cs_row[:], c_sb[:], AF.Silu)
                cols_from_row(cs_row, csT, pscol)

                # ln1 stats: per 512-chunk, sum and sumsq over d
                mu_row = rows.tile([1, N], F32, name='mu_row')
                ex2_row = rows.tile([1, N], F32, name='ex2_row')
                for ch in range(2):
                    sl = slice(ch * NQ, (ch + 1) * NQ)
                    ss = psstat.tile([1, NQ], F32, tag="st_s", name='st_s')
                    sq_ps = psstat.tile([1, NQ], F32, tag="st_q",
                                        name='st_q')
                    for k in range(KT):
                        sq = sqpool.tile([P, NQ], BF16, tag="xsq",
                                         name='xsq')
                        nc.vector.tensor_mul(sq[:], xt[k][:, sl],
                                             xt[k][:, sl])
                        nc.tensor.matmul(ss[:], lhsT=_r(ones_col[:]),
                                         rhs=_r(xt[k][:, sl]),
                                         start=(k == 0), stop=(k == KT - 1))
                        nc.tensor.matmul(sq_ps[:], lhsT=_r(ones_col[:]),
                                         rhs=_r(sq[:]),
                                         start=(k == 0), stop=(k == KT - 1))
                    nc.vector.tensor_scalar_mul(mu_row[0:1, sl], ss[:],
                                                1.0 / D)
                    nc.vector.tensor_scalar_mul(ex2_row[0:1, sl], sq_ps[:],
                                                1.0 / D)

                # mod = silu(c) @ Wada + bada   [1, 6144]
                mod_row = rows.tile([1, 6 * D], BF16, name='mod_row')
                for g in range(12):
                    wch = [wpool.tile([P, NQ], BF16, tag="wada",
                                      name='wada') for _ in range(KT)]
                    for k in range(KT):
                        eng = nc.sync if k % 2 == 0 else nc.gpsimd
                        eng.dma_start(
                            wch[k][:], Wada[k * P:(k + 1) * P,
                                            g * NQ:(g + 1) * NQ])
                    mp = psmod.tile([1, NQ], F32, tag="modps", name='modps')
                    for k in range(KT):
                        nc.tensor.matmul(mp[:], lhsT=_r(csT[:, k:k + 1]),
                                         rhs=_r(wch[k][:]),
                                         start=(k == 0), stop=(k == KT - 1))
                    nc.vector.tensor_add(
                        mod_row[0:1, g * NQ:(g + 1) * NQ], mp[:],
                        bad_full[0:1, g * NQ:(g + 1) * NQ])

                var_row = rows.tile([1, N], F32, name='var_row')
                nc.vector.tensor_mul(var_row[:], mu_row[:], mu_row[:])
                nc.vector.tensor_sub(var_row[:], ex2_row[:], var_row[:])
                a_row = rows.tile([1, N], BF16, name='a_row')
                nc.scalar.activation(a_row[:], var_row[:],
                                     AF.Abs_reciprocal_sqrt, bias=eps_t[:])
                b_row = rows.tile([1, N], BF16, name='b_row')
                nc.vector.scalar_tensor_tensor(
                    b_row[:], mu_row[:], -1.0, a_row[:],
                    ALU.mult, ALU.mult)

                # modulation rows / columns
                S1_row = rows.tile([1, D], BF16, name='S1_row')
                nc.vector.tensor_scalar_add(S1_row[:],
                                            mod_row[0:1, D:2 * D], 1.0)
                nc.vector.tensor_scalar_add(S2_row[:],
                                            mod_row[0:1, 4 * D:5 * D], 1.0)
                nc.vector.tensor_copy(sh2_row_t[:],
                                      mod_row[0:1, 3 * D:4 * D])
                cols_from_row(mod_row[0:1, 2 * D:3 * D], gmsaT, pscol)
                cols_from_row(mod_row[0:1, 5 * D:6 * D], gmlpT, pscol)
                sh1_row = mod_row[0:1, 0:D]

            # h = x*A + B  (A = S1 (x) a, B = S1 (x) b + sh1 (x) 1)
            with ExitStack() as sec:
                psab = sec.enter_context(
                    tc.tile_pool(name="psab", bufs=2, space="PSUM"))
                for k in range(KT):
                    for ch in range(2):
                        sl = slice(ch * NQ, (ch + 1) * NQ)
                        pa = psab.tile([P, NQ], F32, tag="pA", name='pA')
                        pb = psab.tile([P, NQ], F32, tag="pB", name='pB')
                        nc.tensor.matmul(
                            pa[:], lhsT=_r(S1_row[0:1, k * P:(k + 1) * P]),
                            rhs=_r(a_row[0:1, sl]), start=True, stop=True)
                        nc.tensor.matmul(
                            pb[:], lhsT=_r(S1_row[0:1, k * P:(k + 1) * P]),
                            rhs=_r(b_row[0:1, sl]), start=True, stop=False)
                        nc.tensor.matmul(
                            pb[:], lhsT=_r(sh1_row[0:1, k * P:(k + 1) * P]),
                            rhs=_r(ones_row[:]), start=False, stop=True)
                        nc.vector.tensor_mul(hT[k][:, sl], xt[k][:, sl],
                                             pa[:])
                        nc.vector.tensor_add(hT[k][:, sl], hT[k][:, sl],
                                             pb[:])

        # ---------------- phase 2: q, k, v projections ----------------
        qkv_cm = tc.tile_pool(name="qkvp", bufs=1, side='right')
        qkvp = qkv_cm.__enter__()
        qTt = [qkvp.tile([P, NQ], BF16, tag=f"q{k}", name=f"q{k}")
               for k in range(KT)]
        kTt = [qkvp.tile([P, N], BF16, tag=f"k{k}", name=f"k{k}")
               for k in range(KT)]
        vRt = [qkvp.tile([P, H * (DH + 1)], BF16, tag=f"v{k}", name=f"v{k}")
               for k in range(KT)]

        wkv_cm = tc.tile_pool(name="wkvp", bufs=1, side='right')
        wkvp = wkv_cm.__enter__()
        wkc = {}   # (g) -> k-part chunks; ('v', vg) -> v-part chunks
        for g in range(2):
            wkc[g] = [wkvp.tile([P, NQ], BF16, tag=f"kg{g}_{k}",
                                name=f"kg{g}_{k}") for k in range(KT)]
            for k in range(KT):
                eng = nc.sync if k % 2 == 0 else nc.gpsimd
                eng.dma_start(wkc[g][k][:],
                              Wkv[k * P:(k + 1) * P, g * NQ:(g + 1) * NQ])
        for vg in range(2):
            wkc['v', vg] = [wkvp.tile([P, NQ], BF16, tag=f"vg{vg}_{k}",
                                      name=f"vg{vg}_{k}")
                            for k in range(KT)]
            for k in range(KT):
                eng = nc.sync if k % 2 == 0 else nc.gpsimd
                eng.dma_start(wkc['v', vg][k][:],
                              Wkv[k * P:(k + 1) * P,
                                  D + vg * NQ:D + (vg + 1) * NQ])

        prj_cm = tc.tile_pool(name="prjps", bufs=1, space="PSUM",
                              side='right')
        prjps = prj_cm.__enter__()

        def emit_kT(t, ch):
            g, dot = t // 4, t % 4
            sl = slice(ch * NQ, (ch + 1) * NQ)
            p = prjps.tile([P, NQ], F32, tag="prj", name='prj')
            for k in range(KT):
                nc.tensor.matmul(
                    p[:], lhsT=_r(wkc[g][k][:, dot * P:(dot + 1) * P]),
                    rhs=_r(hT[k][:, sl]),
                    start=(k == 0), stop=(k == KT - 1))
            nc.vector.tensor_scalar_add(kTt[t][:, sl], p[:],
                                        bkT[:, t:t + 1])

        def emit_v(vg, nt):
            p = prjps.tile([P, NQ], F32, tag="prj", name='prj')
            for k in range(KT):
                nc.tensor.matmul(
                    p[:], lhsT=_r(hT[k][:, nt * P:(nt + 1) * P]),
                    rhs=_r(wkc['v', vg][k][:]),
                    start=(k == 0), stop=(k == KT - 1))
            vv = vRt[nt].rearrange("p (h w) -> p h w", w=DH + 1)
            pv = p.rearrange("p (h w) -> p h w", w=DH)
            nc.vector.tensor_copy(vv[:, vg * 8:(vg + 1) * 8, 0:DH], pv[:])

        with ExitStack() as ph:
            wpool = ph.enter_context(tc.tile_pool(name="p2w", bufs=26))
            ps = ph.enter_context(
                tc.tile_pool(name="p2ps", bufs=6, space="PSUM"))

            for nt in range(KT):
                vv = vRt[nt].rearrange("p (h w) -> p h w", w=DH + 1)
                nc.vector.memset(vv[:, :, DH:DH + 1], 1.0)

            def stationary_group(wdram, col0, movs, evict, tagp):
                wch = [wpool.tile([P, NQ], BF16, tag=tagp, name=tagp)
                       for _ in range(KT)]
                for k in range(KT):
                    eng = nc.sync if k % 2 == 0 else nc.gpsimd
                    eng.dma_start(
                        wch[k][:], wdram[k * P:(k + 1) * P, col0:col0 + NQ])
                for dot in range(4):
                    p = ps.tile([P, NQ], F32, tag="prj", name='prj')
                    for k in range(KT):
                        nc.tensor.matmul(
                            p[:], lhsT=_r(wch[k][:, dot * P:(dot + 1) * P]),
                            rhs=movs[k], start=(k == 0), stop=(k == KT - 1))
                    evict(dot, p)

            # q^T (own rows), scaled by 1/sqrt(DH)
            for g in range(2):
                def ev_q(dot, p, g=g):
                    t = 4 * g + dot
                    nc.vector.tensor_scalar(qTt[t][:], p[:], DH ** -0.5,
                                            bqT_s[:, t:t + 1],
                                            ALU.mult, ALU.add)
                stationary_group(Wq, g * NQ,
                                 [_r(hT[k][:, 0:NQ]) for k in range(KT)],
                                 ev_q, "wst")

            # k^T tiles 0-1 and v-group 0 now; the rest is emitted inside
            # the attention loop as just-in-time full-array work that keeps
            # the PE clock warm
            for t in range(2):
                emit_kT(t, 0)
                emit_kT(t, 1)
            for nt in range(KT):
                emit_v(0, nt)

        # ---------------- phase 3: attention ----------------

        with ExitStack() as ph:
            epool = ph.enter_context(tc.tile_pool(name="p3e", bufs=52))
            spool = ph.enter_context(tc.tile_pool(name="p3s", bufs=4))
            ps_sim = ph.enter_context(
                tc.tile_pool(name="ps_sim", bufs=3, space="PSUM"))
            ps_bc = ph.enter_context(
                tc.tile_pool(name="ps_bc", bufs=1, space="PSUM"))
            ps_o = ph.enter_context(
                tc.tile_pool(name="ps_o", bufs=3, space="PSUM"))

            for hp in range(H // 2):
                pt = hp
                if 0 < hp < 7:
                    emit_kT(hp + 1, 0)
                    emit_kT(hp + 1, 1)
                if hp < 4:
                    emit_v(1, 2 * hp)
                    emit_v(1, 2 * hp + 1)
                et = {0: [], 1: []}
                for kt in range(KT):
                    pp = {}
                    for hi in range(2):
                        hh = hi * DH
                        p = ps_sim.tile([P, NQ], F32, tag="sim",
                                        name='sim')
                        nc.tensor.matmul(
                            p[:],
                            lhsT=_r(kTt[pt][hh:hh + DH,
                                            kt * P:(kt + 1) * P]),
                            rhs=_r(qTt[pt][hh:hh + DH, :]),
                            start=True, stop=True)
                        pp[hi] = p
                    for hi in range(2):
                        e = epool.tile([P, NQ], BF16, tag="e", name='e')
                        nc.scalar.activation(e[:], pp[hi][:], AF.Exp)
                        et[hi].append(e)
                pos = {}
                for hi in range(2):
                    pos[hi] = ps_o.tile([DH + 1, NQ], F32, tag="ov",
                                        name='ov')
                for kt in range(KT):
                    for hi in range(2):
                        h = 2 * hp + hi
                        nc.tensor.matmul(
                            pos[hi][:],
                            lhsT=_r(vRt[kt][:, h * (DH + 1):
                                            (h + 1) * (DH + 1)]),
                            rhs=_r(et[hi][kt][:]),
                            start=(kt == 0), stop=(kt == KT - 1))
                for hi in range(2):
                    hh = hi * DH
                    po = pos[hi]
                    rf = spool.tile([DH + 1, NQ], F32, tag="rf", name='rf')
                    nc.vector.reciprocal_approx_fast(rf[:], po[:])
                    inv_s = spool.tile([1, NQ], BF16, tag="invs",
                                       name='invs')
                    nc.vector.tensor_copy(inv_s[:], rf[DH:DH + 1, :])
                    pb = ps_bc.tile([DH, NQ], F32, tag="bc", name='bc')
                    nc.tensor.matmul(pb[:], lhsT=_r(ones_row[0:1, 0:DH]),
                                     rhs=_r(inv_s[:]), start=True,
                                     stop=True)
                    binv = spool.tile([DH, NQ], F32, tag="binv",
                                      name='binv')
                    nc.vector.tensor_copy(binv[:], pb[:])
                    nc.vector.tensor_mul(outT[pt][hh:hh + DH, :],
                                         po[0:DH, :], binv[:])

        prj_cm.__exit__(None, None, None)
        wkv_cm.__exit__(None, None, None)
        qkv_cm.__exit__(None, None, None)
        hT_cm.__exit__(None, None, None)

        # ---------------- phase 4: Wo + residual + ln2 ----------------
        x1p = root.enter_context(tc.tile_pool(name="x1p", bufs=1, side='right'))
        x1t = [x1p.tile([P, NQ], BF16, tag=f"x1{k}", name=f"x1{k}")
               for k in range(KT)]
        h2p = root.enter_context(tc.tile_pool(name="h2p", bufs=1, side='right'))
        h2t = [h2p.tile([P, NQ], BF16, tag=f"h2{k}", name=f"h2{k}")
               for k in range(KT)]

        with ExitStack() as ph:
            rows4 = ph.enter_context(tc.tile_pool(name="p4rows", bufs=1))
            wpool = ph.enter_context(tc.tile_pool(name="p4w", bufs=10))
            xpool = ph.enter_context(tc.tile_pool(name="p4x", bufs=1))
            tpool = ph.enter_context(tc.tile_pool(name="p4t", bufs=3))

            xo = [xpool.tile([P, NQ], BF16, tag=f"xo{k}", name=f"xo{k}")
                  for k in range(KT)]
            for k in range(KT):
                eng = nc.sync if k % 2 == 0 else nc.gpsimd
                eng.dma_start(xo[k][:], xT[k * P:(k + 1) * P, 0:NQ])

            bop_row = rows4.tile([1, D], BF16, name='bop_row')
            boT = const.tile([P, KT], F32, name='boT')
            gboT = const.tile([P, KT], F32, name='gboT')

            with ExitStack() as sec:
                psv = sec.enter_context(
                    tc.tile_pool(name="psv", bufs=2, space="PSUM"))
                pscol2 = sec.enter_context(
                    tc.tile_pool(name="pscol2", bufs=1, space="PSUM"))
                psy = sec.enter_context(
                    tc.tile_pool(name="psy", bufs=2, space="PSUM"))

                for g in range(2):
                    wch = [wpool.tile([P, NQ], BF16, tag="wo", name='wo')
                           for _ in range(KT)]
                    for k in range(KT):
                        eng = nc.sync if k % 2 == 0 else nc.gpsimd
                        eng.dma_start(
                            wch[k][:],
                            Wo[k * P:(k + 1) * P, g * NQ:(g + 1) * NQ])
                    mp = psv.tile([1, NQ], F32, tag="bvps", name='bvps')
                    for k in range(KT):
                        nc.tensor.matmul(mp[:], lhsT=_r(bvT[:, k:k + 1]),
                                         rhs=_r(wch[k][:]),
                                         start=(k == 0), stop=(k == KT - 1))
                    nc.vector.tensor_add(
                        bop_row[0:1, g * NQ:(g + 1) * NQ], mp[:],
                        bo_row[0:1, g * NQ:(g + 1) * NQ])
                    cols_from_row(bop_row[0:1, g * NQ:(g + 1) * NQ],
                                  boT[:, g * 4:(g + 1) * 4], pscol2)
                    nc.vector.tensor_mul(gboT[:, g * 4:(g + 1) * 4],
                                         gmsaT[:, g * 4:(g + 1) * 4],
                                         boT[:, g * 4:(g + 1) * 4])
                    for dot in range(4):
                        t = 4 * g + dot
                        p = psy.tile([P, NQ], F32, tag="y1", name='y1')
                        for k in range(KT):
                            nc.tensor.matmul(
                                p[:],
                                lhsT=_r(wch[k][:, dot * P:(dot + 1) * P]),
                                rhs=_r(outT[k][:]),
                                start=(k == 0), stop=(k == KT - 1))
                        nc.vector.affine_then_add(
                            x1t[t][:], p[:], xo[t][:],
                            scale=gmsaT[:, t:t + 1],
                            bias=gboT[:, t:t + 1])

            with ExitStack() as sec:
                psstat2 = sec.enter_context(
                    tc.tile_pool(name="psstat2", bufs=1, space="PSUM"))
                psab2 = sec.enter_context(
                    tc.tile_pool(name="psab2", bufs=2, space="PSUM"))

                ss = psstat2.tile([1, NQ], F32, tag="st2s", name='st2s')
                sq_ps = psstat2.tile([1, NQ], F32, tag="st2q", name='st2q')
                for k in range(KT):
                    sq = tpool.tile([P, NQ], BF16, tag="x1sq", name='x1sq')
                    nc.vector.tensor_mul(sq[:], x1t[k][:], x1t[k][:])
                    nc.tensor.matmul(ss[:], lhsT=_r(ones_col[:]),
                                     rhs=_r(x1t[k][:]),
                                     start=(k == 0), stop=(k == KT - 1))
                    nc.tensor.matmul(sq_ps[:], lhsT=_r(ones_col[:]),
                                     rhs=_r(sq[:]),
                                     start=(k == 0), stop=(k == KT - 1))
                mu2 = rows4.tile([1, NQ], F32, name='mu2')
                ex22 = rows4.tile([1, NQ], F32, name='ex22')
                nc.vector.tensor_scalar_mul(mu2[:], ss[:], 1.0 / D)
                nc.vector.tensor_scalar_mul(ex22[:], sq_ps[:], 1.0 / D)
                var2 = rows4.tile([1, NQ], F32, name='var2')
                nc.vector.tensor_mul(var2[:], mu2[:], mu2[:])
                nc.vector.tensor_sub(var2[:], ex22[:], var2[:])
                a2 = rows4.tile([1, NQ], BF16, name='a2')
                nc.scalar.activation(a2[:], var2[:],
                                     AF.Abs_reciprocal_sqrt, bias=eps_t[:])
                b2r = rows4.tile([1, NQ], BF16, name='b2r')
                nc.vector.scalar_tensor_tensor(
                    b2r[:], mu2[:], -1.0, a2[:], ALU.mult, ALU.mult)

                for k in range(KT):
                    pa = psab2.tile([P, NQ], F32, tag="pA2", name='pA2')
                    pb = psab2.tile([P, NQ], F32, tag="pB2", name='pB2')
                    nc.tensor.matmul(
                        pa[:], lhsT=_r(S2_row[0:1, k * P:(k + 1) * P]),
                        rhs=_r(a2[:]), start=True, stop=True)
                    nc.tensor.matmul(
                        pb[:], lhsT=_r(S2_row[0:1, k * P:(k + 1) * P]),
                        rhs=_r(b2r[:]), start=True, stop=False)
                    nc.tensor.matmul(
                        pb[:], lhsT=_r(sh2_row_t[0:1, k * P:(k + 1) * P]),
                        rhs=_r(ones_row[:]), start=False, stop=True)
                    nc.vector.tensor_mul(h2t[k][:], x1t[k][:], pa[:])
                    nc.vector.tensor_add(h2t[k][:], h2t[k][:], pb[:])

        op_cm.__exit__(None, None, None)

        # ---------------- phase 5: MLP ----------------
        with ExitStack() as ph:
            gp = ph.enter_context(tc.tile_pool(name="gp", bufs=1))
            gTt = [gp.tile([P, NQ], BF16, tag=f"g{m}", name=f"g{m}")
                   for m in range(MT)]
            wpool = ph.enter_context(tc.tile_pool(name="p5w", bufs=24))
            w2pool = ph.enter_context(tc.tile_pool(name="p5w2", bufs=16))
            opool = ph.enter_context(tc.tile_pool(name="p5o", bufs=3))
            ps1 = ph.enter_context(
                tc.tile_pool(name="ps1", bufs=4, space="PSUM"))
            ps2 = ph.enter_context(
                tc.tile_pool(name="ps2", bufs=1, space="PSUM"))

            g2b2T = const.tile([P, KT], F32, name='g2b2T')
            nc.vector.tensor_mul(g2b2T[:], gmlpT[:], b2T[:])

            for g in range(MLPD // NQ):   # 8 column groups
                wch = [wpool.tile([P, NQ], BF16, tag="w1", name='w1')
                       for _ in range(KT)]
                for k in range(KT):
                    eng = nc.sync if k % 2 == 0 else nc.gpsimd
                    eng.dma_start(
                        wch[k][:], W1[k * P:(k + 1) * P,
                                      g * NQ:(g + 1) * NQ])
                for dot in range(4):
                    m = 4 * g + dot
                    p = ps1.tile([P, NQ], F32, tag="m1", name='m1')
                    for k in range(KT):
                        nc.tensor.matmul(
                            p[:], lhsT=_r(wch[k][:, dot * P:(dot + 1) * P]),
                            rhs=_r(h2t[k][:]),
                            start=(k == 0), stop=(k == KT - 1))
                    nc.scalar.activation(gTt[m][:], p[:], AF.Gelu_apprx_tanh,
                                         bias=b1T[:, m:m + 1])

            for half in range(2):
                pacc = [ps2.tile([P, NQ], F32, tag=f"acc{d}",
                                 name=f"acc{d}") for d in range(4)]
                for mk in range(MT):
                    w2c = w2pool.tile([P, NQ], BF16, tag="w2", name='w2')
                    eng = nc.sync if mk % 2 == 0 else nc.gpsimd
                    eng.dma_start(
                        w2c[:], W2[mk * P:(mk + 1) * P,
                                   half * NQ:(half + 1) * NQ])
                    for d in range(4):
                        nc.tensor.matmul(
                            pacc[d][:],
                            lhsT=_r(w2c[:, d * P:(d + 1) * P]),
                            rhs=_r(gTt[mk][:]),
                            start=(mk == 0), stop=(mk == MT - 1))
                for d in range(4):
                    t = half * 4 + d
                    yt = opool.tile([P, NQ], BF16, tag="yout", name='yout')
                    nc.vector.affine_then_add(
                        yt[:], pacc[d][:], x1t[t][:],
                        scale=gmlpT[:, t:t + 1], bias=g2b2T[:, t:t + 1])
                    eng = nc.sync if t % 2 == 0 else nc.gpsimd
                    eng.dma_start(yT[t * P:(t + 1) * P, :], yt[:])

    nc.compile()
    return nc


_NC = None


def _get_nc():
    global _NC
    if _NC is None:
        _NC = build()
    return _NC


def _prep_inputs(x, c, Wq, bq, Wkv, bkv, Wo, bo, W1, b1, W2, b2, Wada, bada):
    import ml_dtypes
    f = np.float32
    bf = ml_dtypes.bfloat16
    col = lambda v, n: np.ascontiguousarray(
        np.asarray(v, f).reshape(n, P).T)
    shared = {
        "Wq": np.asarray(Wq, f).astype(bf), "Wkv": np.asarray(Wkv, f).astype(bf),
        "Wo": np.asarray(Wo, f).astype(bf), "W1": np.asarray(W1, f).astype(bf),
        "W2": np.asarray(W2, f).astype(bf), "Wada": np.asarray(Wada, f).astype(bf),
        "bada_r": np.asarray(bada, f).reshape(1, -1),
        "bq_c": col(bq, KT), "bk_c": col(np.asarray(bkv, f)[:D], KT),
        "bv_c": col(np.asarray(bkv, f)[D:], KT).astype(bf),
        "bo_r": np.asarray(bo, f).reshape(1, -1),
        "b1_c": col(b1, MT), "b2_c": col(b2, KT),
    }
    in_maps = []
    for core in range(NCORES):
        b, half = core // 2, core % 2
        xb = np.asarray(x[b], f)
        perm = np.concatenate(
            [xb[half * NQ:(half + 1) * NQ],
             xb[(1 - half) * NQ:(2 - half) * NQ]], axis=0)
        m = dict(shared)
        m["xT"] = np.ascontiguousarray(perm.T).astype(bf)
        m["crow"] = np.asarray(c[b:b + 1], f)
        in_maps.append(m)
    return in_maps


def _run(inputs, trace=False):
    nc = _get_nc()
    in_maps = _prep_inputs(**inputs)
    res = run_bass_kernel_spmd(nc, in_maps, core_ids=list(range(NCORES)),
                               trace=trace)
    B = 4
    y = np.empty((B, N, D), np.float32)
    for core in range(NCORES):
        b, half = core // 2, core % 2
        y[b, half * NQ:(half + 1) * NQ, :] = (
            res.results[core]["yT"].astype(np.float32).T)
    return y, res


def kernel(**inputs):
    y, _ = _run(inputs, trace=False)
    return y



# revision 23
# speedup vs baseline: 1.3729x; 1.0011x over previous
"""AdaLN attention block (DiT-style) on 8 TRN2 NeuronCores.

Sharding: 8 cores = 4 batches x 2 token-halves, no collectives. Core c handles
batch c//2 and query-token half c%2: layernorm1 and k/v are computed over the
full (permuted) sequence, everything else only for the own 512 query rows.

Device layout is feature-major (activations transposed, [d, n]). X @ W runs
with W column-tiles stationary and X^T moving, producing Y^T directly.
LayerNorm statistics use ones-vector matmuls (partition-axis sums on the PE);
the AdaLN modulate is h = x*A + B with rank-1 A/B built by K=1 outer-product
matmuls into PSUM. Softmax skips max-subtraction (fp32 exp is safe for this
distribution); the denominator is a ones-column appended to the attn@v
stationary operand; normalization is folded in per head via a broadcast
reciprocal.
"""

import numpy as np
from contextlib import ExitStack

import concourse.bass as bass
import concourse.bacc as bacc
import concourse.mybir as mybir
from concourse import tile
from concourse.bass_utils import run_bass_kernel_spmd

P = 128
D = 1024
N = 1024
NQ = 512
H = 16
DH = 64
MLPD = 4096
EPS = 1e-6
NCORES = 8

F32 = mybir.dt.float32
BF16 = mybir.dt.bfloat16
AF = mybir.ActivationFunctionType
ALU = mybir.AluOpType

KT = D // P           # 8 contraction tiles over D
MT = MLPD // P        # 32 tiles over MLP dim


def _r(ap):
    return ap


def build():
    nc = bacc.Bacc("TRN2", target_bir_lowering=False, debug=False,
                   num_devices=NCORES)

    xT = nc.dram_tensor("xT", [D, N], BF16, kind="ExternalInput")
    crow = nc.dram_tensor("crow", [1, D], F32, kind="ExternalInput")
    Wq = nc.dram_tensor("Wq", [D, D], BF16, kind="ExternalInput")
    Wkv = nc.dram_tensor("Wkv", [D, 2 * D], BF16, kind="ExternalInput")
    Wo = nc.dram_tensor("Wo", [D, D], BF16, kind="ExternalInput")
    W1 = nc.dram_tensor("W1", [D, MLPD], BF16, kind="ExternalInput")
    W2 = nc.dram_tensor("W2", [MLPD, D], BF16, kind="ExternalInput")
    Wada = nc.dram_tensor("Wada", [D, 6 * D], BF16, kind="ExternalInput")
    bada_r = nc.dram_tensor("bada_r", [1, 6 * D], F32, kind="ExternalInput")
    bq_c = nc.dram_tensor("bq_c", [P, KT], F32, kind="ExternalInput")
    bk_c = nc.dram_tensor("bk_c", [P, KT], F32, kind="ExternalInput")
    bv_c = nc.dram_tensor("bv_c", [P, KT], BF16, kind="ExternalInput")
    bo_r = nc.dram_tensor("bo_r", [1, D], F32, kind="ExternalInput")
    b1_c = nc.dram_tensor("b1_c", [P, MT], F32, kind="ExternalInput")
    b2_c = nc.dram_tensor("b2_c", [P, KT], F32, kind="ExternalInput")
    yT = nc.dram_tensor("yT", [D, NQ], BF16, kind="ExternalOutput")

    with tile.TileContext(nc) as tc, ExitStack() as root:
        const = root.enter_context(tc.tile_pool(name="const", bufs=1))
        rootrows = root.enter_context(tc.tile_pool(name="rootrows", bufs=1))

        ones_col = const.tile([P, 1], BF16, name='ones_col')
        nc.vector.memset(ones_col[:], 1.0)
        ones_col_f = const.tile([P, 1], F32, name='ones_col_f')
        nc.vector.memset(ones_col_f[:], 1.0)
        ones_row = const.tile([1, NQ], BF16, name='ones_row')
        nc.vector.memset(ones_row[:], 1.0)
        eps_t = const.tile([1, 1], F32, name='eps_t')
        nc.vector.memset(eps_t[:], EPS)

        bqT = const.tile([P, KT], F32, name='bqT')
        nc.sync.dma_start(bqT[:], bq_c[:])
        bkT = const.tile([P, KT], F32, name='bkT')
        nc.sync.dma_start(bkT[:], bk_c[:])
        bvT = const.tile([P, KT], BF16, name='bvT')
        nc.sync.dma_start(bvT[:], bv_c[:])
        b1T = const.tile([P, MT], F32, name='b1T')
        nc.sync.dma_start(b1T[:], b1_c[:])
        b2T = const.tile([P, KT], F32, name='b2T')
        nc.sync.dma_start(b2T[:], b2_c[:])
        bo_row = const.tile([1, D], F32, name='bo_row')
        nc.sync.dma_start(bo_row[:], bo_r[:])

        bqT_s = const.tile([P, KT], F32, name='bqT_s')
        nc.vector.tensor_scalar_mul(bqT_s[:], bqT[:], DH ** -0.5)

        def cols_from_row(row_ap, dst, psum_pool, plus1=False):
            """[1, n*128] row -> [128, n] column tile via K=1 matmuls."""
            n = dst.shape[-1]
            ps = psum_pool.tile([P, n], F32, tag="colps", name='colps')
            for j in range(n):
                nc.tensor.matmul(ps[:, j:j + 1],
                                 lhsT=_r(row_ap[0:1, j * P:(j + 1) * P]),
                                 rhs=_r(ones_row[0:1, 0:1]),
                                 start=True, stop=True)
            if plus1:
                nc.vector.tensor_scalar_add(dst[:], ps[:], 1.0)
            else:
                nc.vector.tensor_copy(dst[:], ps[:])
            return dst

        csT = const.tile([P, KT], BF16, name='csT')
        gmsaT = const.tile([P, KT], F32, name='gmsaT')
        gmlpT = const.tile([P, KT], F32, name='gmlpT')
        S1T = const.tile([P, KT], F32, name='S1T')
        sh1T = const.tile([P, KT], F32, name='sh1T')
        S2T = const.tile([P, KT], F32, name='S2T')
        sh2T = const.tile([P, KT], F32, name='sh2T')

        # persistent activation arrays (distinct tag per tile, 1 buf each)
        op_cm = tc.tile_pool(name="op", bufs=1, side='left')
        op_ = op_cm.__enter__()
        outT = [op_.tile([P, NQ], BF16, tag=f"o{k}", name=f"o{k}")
                for k in range(KT)]
        hT_cm = tc.tile_pool(name="hTp", bufs=1, side='left')
        hTp = hT_cm.__enter__()
        hT = [hTp.tile([P, N], BF16, tag=f"h{k}", name=f"h{k}")
              for k in range(KT)]

        # ---------------- phase 0+1: mod vector & ln1 ----------------
        with ExitStack() as ph:
            rows = ph.enter_context(tc.tile_pool(name="p0rows", bufs=1))
            xpool = ph.enter_context(tc.tile_pool(name="p0x", bufs=1))
            sqpool = ph.enter_context(tc.tile_pool(name="p0sq", bufs=4))
            wpool = ph.enter_context(tc.tile_pool(name="p0w", bufs=26))

            # c + bada first, on the scalar HWDGE queue so they are not
            # stuck behind the bulk x/Wada transfers
            c_sb = rows.tile([1, D], F32, name='c_sb')
            nc.scalar.dma_start(c_sb[:], crow[:])
            bad_full = rows.tile([1, 6 * D], F32, name='bad_full')
            nc.scalar.dma_start(bad_full[:], bada_r[:])

            xt = [xpool.tile([P, N], BF16, tag=f"x{k}", name=f"x{k}")
                  for k in range(KT)]
            for k in range(KT):
                eng = nc.sync if k % 2 == 0 else nc.gpsimd
                eng.dma_start(xt[k][:], xT[k * P:(k + 1) * P, :])

            with ExitStack() as sec:
                pscol = sec.enter_context(
                    tc.tile_pool(name="pscol", bufs=1, space="PSUM"))
                psmod = sec.enter_context(
                    tc.tile_pool(name="psmod", bufs=3, space="PSUM"))
                psstat = sec.enter_context(
                    tc.tile_pool(name="psstat", bufs=2, space="PSUM"))

                # silu(c) and its column layout
                cs_row = rows.tile([1, D], BF16, name='cs_row')
                nc.scalar.activation(cs_row[:], c_sb[:], AF.Silu)
                cols_from_row(cs_row, csT, pscol)

                # ln1 stats: per 512-chunk, sum and sumsq over d.
                # sum chains land on col-strip 1 (partition 32), sumsq on
                # strip 2 (partition 64): the PE runs them concurrently.
                mu_row = rows.tile([1, N], F32, name='mu_row')
                ex2_row = rows.tile([1, N], F32, name='ex2_row')
                for ch in range(2):
                    sl = slice(ch * NQ, (ch + 1) * NQ)
                    ss = psstat.tile([P, NQ], F32, tag="st_s", name='st_s')
                    sq_ps = psstat.tile([P, NQ], F32, tag="st_q",
                                        name='st_q')
                    for k in range(KT):
                        sq = sqpool.tile([P, NQ], BF16, tag="xsq",
                                         name='xsq')
                        nc.vector.tensor_mul(sq[:], xt[k][:, sl],
                                             xt[k][:, sl])
                        nc.tensor.matmul(ss[32:33, :],
                                         lhsT=_r(ones_col[:]),
                                         rhs=_r(xt[k][:, sl]),
                                         start=(k == 0), stop=(k == KT - 1))
                        nc.tensor.matmul(sq_ps[64:65, :],
                                         lhsT=_r(ones_col[:]),
                                         rhs=_r(sq[:]),
                                         start=(k == 0), stop=(k == KT - 1))
                    nc.vector.tensor_scalar_mul(mu_row[0:1, sl],
                                                ss[32:33, :], 1.0 / D)
                    nc.vector.tensor_scalar_mul(ex2_row[0:1, sl],
                                                sq_ps[64:65, :], 1.0 / D)

                # mod = silu(c) @ Wada + bada   [1, 6144]
                # groups rotate over col-strips 0/1/2 for 3x concurrency
                mod_row = rows.tile([1, 6 * D], BF16, name='mod_row')
                for g2 in range(6):
                    wch = [wpool.tile([P, 2 * NQ], BF16, tag="wada",
                                      name='wada') for _ in range(KT)]
                    for k in range(KT):
                        eng = nc.sync if k % 2 == 0 else nc.gpsimd
                        eng.dma_start(
                            wch[k][:], Wada[k * P:(k + 1) * P,
                                            g2 * D:(g2 + 1) * D])
                    for sub in range(2):
                        g = 2 * g2 + sub
                        sp = 32 * (g % 3)
                        mp = psmod.tile([P, NQ], F32, tag="modps",
                                        name='modps')
                        for k in range(KT):
                            nc.tensor.matmul(
                                mp[sp:sp + 1, :],
                                lhsT=_r(csT[:, k:k + 1]),
                                rhs=_r(wch[k][:, sub * NQ:(sub + 1) * NQ]),
                                start=(k == 0), stop=(k == KT - 1))
                        nc.vector.tensor_add(
                            mod_row[0:1, g * NQ:(g + 1) * NQ],
                            mp[sp:sp + 1, :],
                            bad_full[0:1, g * NQ:(g + 1) * NQ])

                var_row = rows.tile([1, N], F32, name='var_row')
                nc.vector.tensor_mul(var_row[:], mu_row[:], mu_row[:])
                nc.vector.tensor_sub(var_row[:], ex2_row[:], var_row[:])
                a_row = rows.tile([1, N], BF16, name='a_row')
                nc.scalar.activation(a_row[:], var_row[:],
                                     AF.Abs_reciprocal_sqrt, bias=eps_t[:])
                b_row = rows.tile([1, N], BF16, name='b_row')
                nc.vector.scalar_tensor_tensor(
                    b_row[:], mu_row[:], -1.0, a_row[:],
                    ALU.mult, ALU.mult)

                # modulation columns
                cols_from_row(mod_row[0:1, 0:D], sh1T, pscol)
                cols_from_row(mod_row[0:1, D:2 * D], S1T, pscol,
                              plus1=True)
                cols_from_row(mod_row[0:1, 2 * D:3 * D], gmsaT, pscol)
                cols_from_row(mod_row[0:1, 3 * D:4 * D], sh2T, pscol)
                cols_from_row(mod_row[0:1, 4 * D:5 * D], S2T, pscol,
                              plus1=True)
                cols_from_row(mod_row[0:1, 5 * D:6 * D], gmlpT, pscol)

            # h = (x*S1*a_bcast) + (b_bcast*S1 + sh1): two fused DVE ops
            # per chunk against shared rank-1 broadcast tiles.
            with ExitStack() as sec:
                psbr = sec.enter_context(
                    tc.tile_pool(name="psbr", bufs=1, space="PSUM"))
                ba = {}
                bb = {}
                for ch in range(2):
                    sl = slice(ch * NQ, (ch + 1) * NQ)
                    ba[ch] = psbr.tile([P, NQ], F32, tag=f"ba{ch}",
                                       name=f"ba{ch}")
                    nc.tensor.matmul(ba[ch][:],
                                     lhsT=_r(ones_row[0:1, 0:P]),
                                     rhs=_r(a_row[0:1, sl]),
                                     start=True, stop=True)
                    bb[ch] = psbr.tile([P, NQ], F32, tag=f"bb{ch}",
                                       name=f"bb{ch}")
                    nc.tensor.matmul(bb[ch][:],
                                     lhsT=_r(ones_row[0:1, 0:P]),
                                     rhs=_r(b_row[0:1, sl]),
                                     start=True, stop=True)
                for k in range(KT):
                    for ch in range(2):
                        sl = slice(ch * NQ, (ch + 1) * NQ)
                        t1 = sqpool.tile([P, NQ], BF16, tag="t1",
                                         name='t1')
                        nc.vector.scalar_tensor_tensor(
                            t1[:], xt[k][:, sl], S1T[:, k:k + 1],
                            ba[ch][:], ALU.mult, ALU.mult)
                        nc.vector.affine_then_add(
                            hT[k][:, sl], bb[ch][:], t1[:],
                            scale=S1T[:, k:k + 1], bias=sh1T[:, k:k + 1])

        # ---------------- phase 2: q, k, v projections ----------------
        qkv_cm = tc.tile_pool(name="qkvp", bufs=1, side='right')
        qkvp = qkv_cm.__enter__()
        qTt = [qkvp.tile([P, NQ], BF16, tag=f"q{k}", name=f"q{k}")
               for k in range(KT)]
        kTt = [qkvp.tile([P, N], BF16, tag=f"k{k}", name=f"k{k}")
               for k in range(KT)]
        vRt = [qkvp.tile([P, H * (DH + 1)], BF16, tag=f"v{k}", name=f"v{k}")
               for k in range(KT)]

        wkv_cm = tc.tile_pool(name="wkvp", bufs=1, side='right')
        wkvp = wkv_cm.__enter__()
        wkc = {}   # (g) -> k-part chunks; ('v', vg) -> v-part chunks
        for g in range(2):
            wkc[g] = [wkvp.tile([P, NQ], BF16, tag=f"kg{g}_{k}",
                                name=f"kg{g}_{k}") for k in range(KT)]
            for k in range(KT):
                eng = nc.sync if k % 2 == 0 else nc.gpsimd
                eng.dma_start(wkc[g][k][:],
                              Wkv[k * P:(k + 1) * P, g * NQ:(g + 1) * NQ])
        for vg in range(2):
            wkc['v', vg] = [wkvp.tile([P, NQ], BF16, tag=f"vg{vg}_{k}",
                                      name=f"vg{vg}_{k}")
                            for k in range(KT)]
            for k in range(KT):
                eng = nc.sync if k % 2 == 0 else nc.gpsimd
                eng.dma_start(wkc['v', vg][k][:],
                              Wkv[k * P:(k + 1) * P,
                                  D + vg * NQ:D + (vg + 1) * NQ])

        prj_cm = tc.tile_pool(name="prjps", bufs=1, space="PSUM",
                              side='right')
        prjps = prj_cm.__enter__()

        def emit_kT(t, ch):
            g, dot = t // 4, t % 4
            sl = slice(ch * NQ, (ch + 1) * NQ)
            p = prjps.tile([P, NQ], F32, tag="prj", name='prj')
            for k in range(KT):
                nc.tensor.matmul(
                    p[:], lhsT=_r(wkc[g][k][:, dot * P:(dot + 1) * P]),
                    rhs=_r(hT[k][:, sl]),
                    start=(k == 0), stop=(k == KT - 1))
            nc.vector.tensor_scalar_add(kTt[t][:, sl], p[:],
                                        bkT[:, t:t + 1])

        def emit_v(vg, nt):
            p = prjps.tile([P, NQ], F32, tag="prj", name='prj')
            for k in range(KT):
                nc.tensor.matmul(
                    p[:], lhsT=_r(hT[k][:, nt * P:(nt + 1) * P]),
                    rhs=_r(wkc['v', vg][k][:]),
                    start=(k == 0), stop=(k == KT - 1))
            vv = vRt[nt].rearrange("p (h w) -> p h w", w=DH + 1)
            pv = p.rearrange("p (h w) -> p h w", w=DH)
            nc.vector.tensor_copy(vv[:, vg * 8:(vg + 1) * 8, 0:DH], pv[:])

        with ExitStack() as ph:
            wpool = ph.enter_context(tc.tile_pool(name="p2w", bufs=26))
            ps = ph.enter_context(
                tc.tile_pool(name="p2ps", bufs=6, space="PSUM"))

            for nt in range(KT):
                vv = vRt[nt].rearrange("p (h w) -> p h w", w=DH + 1)
                nc.vector.memset(vv[:, :, DH:DH + 1], 1.0)

            def stationary_group(wdram, col0, movs, evict, tagp):
                wch = [wpool.tile([P, NQ], BF16, tag=tagp, name=tagp)
                       for _ in range(KT)]
                for k in range(KT):
                    eng = nc.sync if k % 2 == 0 else nc.gpsimd
                    eng.dma_start(
                        wch[k][:], wdram[k * P:(k + 1) * P, col0:col0 + NQ])
                for dot in range(4):
                    p = ps.tile([P, NQ], F32, tag="prj", name='prj')
                    for k in range(KT):
                        nc.tensor.matmul(
                            p[:], lhsT=_r(wch[k][:, dot * P:(dot + 1) * P]),
                            rhs=movs[k], start=(k == 0), stop=(k == KT - 1))
                    evict(dot, p)

            # q^T (own rows), scaled by 1/sqrt(DH)
            for g in range(2):
                def ev_q(dot, p, g=g):
                    t = 4 * g + dot
                    nc.vector.tensor_scalar(qTt[t][:], p[:], DH ** -0.5,
                                            bqT_s[:, t:t + 1],
                                            ALU.mult, ALU.add)
                stationary_group(Wq, g * NQ,
                                 [_r(hT[k][:, 0:NQ]) for k in range(KT)],
                                 ev_q, "wst")

            # k^T tiles 0-1 and v-group 0 now; the rest is emitted inside
            # the attention loop as just-in-time full-array work that keeps
            # the PE clock warm
            for t in range(2):
                emit_kT(t, 0)
                emit_kT(t, 1)
            for nt in range(KT):
                emit_v(0, nt)

        # ---------------- phase 3: attention ----------------

        with ExitStack() as ph:
            epool = ph.enter_context(tc.tile_pool(name="p3e", bufs=52))
            spool = ph.enter_context(tc.tile_pool(name="p3s", bufs=4))
            ps_sim = ph.enter_context(
                tc.tile_pool(name="ps_sim", bufs=3, space="PSUM"))
            ps_bc = ph.enter_context(
                tc.tile_pool(name="ps_bc", bufs=1, space="PSUM"))
            ps_o = ph.enter_context(
                tc.tile_pool(name="ps_o", bufs=3, space="PSUM"))

            for hp in range(H // 2):
                pt = hp
                if 0 < hp < 7:
                    emit_kT(hp + 1, 0)
                    emit_kT(hp + 1, 1)
                if hp < 4:
                    emit_v(1, 2 * hp)
                    emit_v(1, 2 * hp + 1)
                et = {0: [], 1: []}
                for kt in range(KT):
                    pp = {}
                    for hi in range(2):
                        hh = hi * DH
                        p = ps_sim.tile([P, NQ], F32, tag="sim",
                                        name='sim')
                        nc.tensor.matmul(
                            p[:],
                            lhsT=_r(kTt[pt][hh:hh + DH,
                                            kt * P:(kt + 1) * P]),
                            rhs=_r(qTt[pt][hh:hh + DH, :]),
                            start=True, stop=True)
                        pp[hi] = p
                    for hi in range(2):
                        e = epool.tile([P, NQ], BF16, tag="e", name='e')
                        nc.scalar.activation(e[:], pp[hi][:], AF.Exp)
                        et[hi].append(e)
                pos = {}
                for hi in range(2):
                    pos[hi] = ps_o.tile([DH + 1, NQ], F32, tag="ov",
                                        name='ov')
                for kt in range(KT):
                    for hi in range(2):
                        h = 2 * hp + hi
                        nc.tensor.matmul(
                            pos[hi][:],
                            lhsT=_r(vRt[kt][:, h * (DH + 1):
                                            (h + 1) * (DH + 1)]),
                            rhs=_r(et[hi][kt][:]),
                            start=(kt == 0), stop=(kt == KT - 1))
                for hi in range(2):
                    hh = hi * DH
                    po = pos[hi]
                    rf = spool.tile([DH + 1, NQ], F32, tag="rf", name='rf')
                    nc.vector.reciprocal_approx_fast(rf[:], po[:])
                    inv_s = spool.tile([1, NQ], BF16, tag="invs",
                                       name='invs')
                    nc.vector.tensor_copy(inv_s[:], rf[DH:DH + 1, :])
                    pb = ps_bc.tile([DH, NQ], F32, tag="bc", name='bc')
                    nc.tensor.matmul(pb[:], lhsT=_r(ones_row[0:1, 0:DH]),
                                     rhs=_r(inv_s[:]), start=True,
                                     stop=True)
                    binv = spool.tile([DH, NQ], F32, tag="binv",
                                      name='binv')
                    nc.vector.tensor_copy(binv[:], pb[:])
                    nc.vector.tensor_mul(outT[pt][hh:hh + DH, :],
                                         po[0:DH, :], binv[:])

        prj_cm.__exit__(None, None, None)
        wkv_cm.__exit__(None, None, None)
        qkv_cm.__exit__(None, None, None)
        hT_cm.__exit__(None, None, None)

        # ---------------- phase 4: Wo + residual + ln2 ----------------
        x1p = root.enter_context(tc.tile_pool(name="x1p", bufs=1, side='right'))
        x1t = [x1p.tile([P, NQ], BF16, tag=f"x1{k}", name=f"x1{k}")
               for k in range(KT)]
        h2p = root.enter_context(tc.tile_pool(name="h2p", bufs=1, side='right'))
        h2t = [h2p.tile([P, NQ], BF16, tag=f"h2{k}", name=f"h2{k}")
               for k in range(KT)]

        with ExitStack() as ph:
            rows4 = ph.enter_context(tc.tile_pool(name="p4rows", bufs=1))
            wpool = ph.enter_context(tc.tile_pool(name="p4w", bufs=10))
            xpool = ph.enter_context(tc.tile_pool(name="p4x", bufs=1))
            tpool = ph.enter_context(tc.tile_pool(name="p4t", bufs=3))

            xo = [xpool.tile([P, NQ], BF16, tag=f"xo{k}", name=f"xo{k}")
                  for k in range(KT)]
            for k in range(KT):
                eng = nc.sync if k % 2 == 0 else nc.gpsimd
                eng.dma_start(xo[k][:], xT[k * P:(k + 1) * P, 0:NQ])

            bop_row = rows4.tile([1, D], BF16, name='bop_row')
            boT = const.tile([P, KT], F32, name='boT')
            gboT = const.tile([P, KT], F32, name='gboT')

            with ExitStack() as sec:
                psv = sec.enter_context(
                    tc.tile_pool(name="psv", bufs=2, space="PSUM"))
                pscol2 = sec.enter_context(
                    tc.tile_pool(name="pscol2", bufs=1, space="PSUM"))
                psy = sec.enter_context(
                    tc.tile_pool(name="psy", bufs=2, space="PSUM"))

                for g in range(2):
                    wch = [wpool.tile([P, NQ], BF16, tag="wo", name='wo')
                           for _ in range(KT)]
                    for k in range(KT):
                        eng = nc.sync if k % 2 == 0 else nc.gpsimd
                        eng.dma_start(
                            wch[k][:],
                            Wo[k * P:(k + 1) * P, g * NQ:(g + 1) * NQ])
                    sp = 32 * (g + 1)
                    mp = psv.tile([P, NQ], F32, tag="bvps", name='bvps')
                    for k in range(KT):
                        nc.tensor.matmul(mp[sp:sp + 1, :],
                                         lhsT=_r(bvT[:, k:k + 1]),
                                         rhs=_r(wch[k][:]),
                                         start=(k == 0), stop=(k == KT - 1))
                    nc.vector.tensor_add(
                        bop_row[0:1, g * NQ:(g + 1) * NQ], mp[sp:sp + 1, :],
                        bo_row[0:1, g * NQ:(g + 1) * NQ])
                    cols_from_row(bop_row[0:1, g * NQ:(g + 1) * NQ],
                                  boT[:, g * 4:(g + 1) * 4], pscol2)
                    nc.vector.tensor_mul(gboT[:, g * 4:(g + 1) * 4],
                                         gmsaT[:, g * 4:(g + 1) * 4],
                                         boT[:, g * 4:(g + 1) * 4])
                    for dot in range(4):
                        t = 4 * g + dot
                        p = psy.tile([P, NQ], F32, tag="y1", name='y1')
                        for k in range(KT):
                            nc.tensor.matmul(
                                p[:],
                                lhsT=_r(wch[k][:, dot * P:(dot + 1) * P]),
                                rhs=_r(outT[k][:]),
                                start=(k == 0), stop=(k == KT - 1))
                        nc.vector.affine_then_add(
                            x1t[t][:], p[:], xo[t][:],
                            scale=gmsaT[:, t:t + 1],
                            bias=gboT[:, t:t + 1])

            with ExitStack() as sec:
                psstat2 = sec.enter_context(
                    tc.tile_pool(name="psstat2", bufs=1, space="PSUM"))
                psbr2 = sec.enter_context(
                    tc.tile_pool(name="psbr2", bufs=1, space="PSUM"))

                ss = psstat2.tile([P, NQ], F32, tag="st2s", name='st2s')
                sq_ps = psstat2.tile([P, NQ], F32, tag="st2q", name='st2q')
                for k in range(KT):
                    sq = tpool.tile([P, NQ], BF16, tag="x1sq", name='x1sq')
                    nc.vector.tensor_mul(sq[:], x1t[k][:], x1t[k][:])
                    nc.tensor.matmul(ss[32:33, :], lhsT=_r(ones_col[:]),
                                     rhs=_r(x1t[k][:]),
                                     start=(k == 0), stop=(k == KT - 1))
                    nc.tensor.matmul(sq_ps[64:65, :], lhsT=_r(ones_col[:]),
                                     rhs=_r(sq[:]),
                                     start=(k == 0), stop=(k == KT - 1))
                mu2 = rows4.tile([1, NQ], F32, name='mu2')
                ex22 = rows4.tile([1, NQ], F32, name='ex22')
                nc.vector.tensor_scalar_mul(mu2[:], ss[32:33, :], 1.0 / D)
                nc.vector.tensor_scalar_mul(ex22[:], sq_ps[64:65, :],
                                            1.0 / D)
                var2 = rows4.tile([1, NQ], F32, name='var2')
                nc.vector.tensor_mul(var2[:], mu2[:], mu2[:])
                nc.vector.tensor_sub(var2[:], ex22[:], var2[:])
                a2 = rows4.tile([1, NQ], BF16, name='a2')
                nc.scalar.activation(a2[:], var2[:],
                                     AF.Abs_reciprocal_sqrt, bias=eps_t[:])
                b2r = rows4.tile([1, NQ], BF16, name='b2r')
                nc.vector.scalar_tensor_tensor(
                    b2r[:], mu2[:], -1.0, a2[:], ALU.mult, ALU.mult)

                ba2 = psbr2.tile([P, NQ], F32, tag="ba2", name='ba2')
                nc.tensor.matmul(ba2[:], lhsT=_r(ones_row[0:1, 0:P]),
                                 rhs=_r(a2[:]), start=True, stop=True)
                bb2 = psbr2.tile([P, NQ], F32, tag="bb2", name='bb2')
                nc.tensor.matmul(bb2[:], lhsT=_r(ones_row[0:1, 0:P]),
                                 rhs=_r(b2r[:]), start=True, stop=True)
                for k in range(KT):
                    t1 = tpool.tile([P, NQ], BF16, tag="t12", name='t12')
                    nc.vector.scalar_tensor_tensor(
                        t1[:], x1t[k][:], S2T[:, k:k + 1], ba2[:],
                        ALU.mult, ALU.mult)
                    nc.vector.affine_then_add(
                        h2t[k][:], bb2[:], t1[:],
                        scale=S2T[:, k:k + 1], bias=sh2T[:, k:k + 1])

        op_cm.__exit__(None, None, None)

        # ---------------- phase 5: MLP ----------------
        with ExitStack() as ph:
            gp = ph.enter_context(tc.tile_pool(name="gp", bufs=1))
            gTt = [gp.tile([P, NQ], BF16, tag=f"g{m}", name=f"g{m}")
                   for m in range(MT)]
            wpool = ph.enter_context(tc.tile_pool(name="p5w", bufs=24))
            w2pool = ph.enter_context(tc.tile_pool(name="p5w2", bufs=16))
            opool = ph.enter_context(tc.tile_pool(name="p5o", bufs=3))
            ps1 = ph.enter_context(
                tc.tile_pool(name="ps1", bufs=4, space="PSUM"))
            ps2 = ph.enter_context(
                tc.tile_pool(name="ps2", bufs=1, space="PSUM"))

            g2b2T = const.tile([P, KT], F32, name='g2b2T')
            nc.vector.tensor_mul(g2b2T[:], gmlpT[:], b2T[:])

            for g in range(MLPD // NQ):   # 8 column groups
                wch = [wpool.tile([P, NQ], BF16, tag="w1", name='w1')
                       for _ in range(KT)]
                for k in range(KT):
                    eng = nc.sync if k % 2 == 0 else nc.gpsimd
                    eng.dma_start(
                        wch[k][:], W1[k * P:(k + 1) * P,
                                      g * NQ:(g + 1) * NQ])
                for dot in range(4):
                    m = 4 * g + dot
                    p = ps1.tile([P, NQ], F32, tag="m1", name='m1')
                    for k in range(KT):
                        nc.tensor.matmul(
                            p[:], lhsT=_r(wch[k][:, dot * P:(dot + 1) * P]),
                            rhs=_r(h2t[k][:]),
                            start=(k == 0), stop=(k == KT - 1))
                    nc.scalar.activation(gTt[m][:], p[:], AF.Gelu_apprx_tanh,
                                         bias=b1T[:, m:m + 1])

            for half in range(2):
                pacc = [ps2.tile([P, NQ], F32, tag=f"acc{d}",
                                 name=f"acc{d}") for d in range(4)]
                for mk in range(MT):
                    w2c = w2pool.tile([P, NQ], BF16, tag="w2", name='w2')
                    eng = nc.sync if mk % 2 == 0 else nc.gpsimd
                    eng.dma_start(
                        w2c[:], W2[mk * P:(mk + 1) * P,
                                   half * NQ:(half + 1) * NQ])
                    for d in range(4):
                        nc.tensor.matmul(
                            pacc[d][:],
                            lhsT=_r(w2c[:, d * P:(d + 1) * P]),
                            rhs=_r(gTt[mk][:]),
                            start=(mk == 0), stop=(mk == MT - 1))
                for d in range(4):
                    t = half * 4 + d
                    yt = opool.tile([P, NQ], BF16, tag="yout", name='yout')
                    nc.vector.affine_then_add(
                        yt[:], pacc[d][:], x1t[t][:],
                        scale=gmlpT[:, t:t + 1], bias=g2b2T[:, t:t + 1])
                    eng = nc.sync if t % 2 == 0 else nc.gpsimd
                    eng.dma_start(yT[t * P:(t + 1) * P, :], yt[:])

    nc.compile()
    return nc


_NC = None


def _get_nc():
    global _NC
    if _NC is None:
        _NC = build()
    return _NC


def _prep_inputs(x, c, Wq, bq, Wkv, bkv, Wo, bo, W1, b1, W2, b2, Wada, bada):
    import ml_dtypes
    f = np.float32
    bf = ml_dtypes.bfloat16
    col = lambda v, n: np.ascontiguousarray(
        np.asarray(v, f).reshape(n, P).T)
    shared = {
        "Wq": np.asarray(Wq, f).astype(bf), "Wkv": np.asarray(Wkv, f).astype(bf),
        "Wo": np.asarray(Wo, f).astype(bf), "W1": np.asarray(W1, f).astype(bf),
        "W2": np.asarray(W2, f).astype(bf), "Wada": np.asarray(Wada, f).astype(bf),
        "bada_r": np.asarray(bada, f).reshape(1, -1),
        "bq_c": col(bq, KT), "bk_c": col(np.asarray(bkv, f)[:D], KT),
        "bv_c": col(np.asarray(bkv, f)[D:], KT).astype(bf),
        "bo_r": np.asarray(bo, f).reshape(1, -1),
        "b1_c": col(b1, MT), "b2_c": col(b2, KT),
    }
    in_maps = []
    for core in range(NCORES):
        b, half = core // 2, core % 2
        xb = np.asarray(x[b], f)
        perm = np.concatenate(
            [xb[half * NQ:(half + 1) * NQ],
             xb[(1 - half) * NQ:(2 - half) * NQ]], axis=0)
        m = dict(shared)
        m["xT"] = np.ascontiguousarray(perm.T).astype(bf)
        m["crow"] = np.asarray(c[b:b + 1], f)
        in_maps.append(m)
    return in_maps


def _run(inputs, trace=False):
    nc = _get_nc()
    in_maps = _prep_inputs(**inputs)
    res = run_bass_kernel_spmd(nc, in_maps, core_ids=list(range(NCORES)),
                               trace=trace)
    B = 4
    y = np.empty((B, N, D), np.float32)
    for core in range(NCORES):
        b, half = core // 2, core % 2
        y[b, half * NQ:(half + 1) * NQ, :] = (
            res.results[core]["yT"].astype(np.float32).T)
    return y, res


def kernel(**inputs):
    y, _ = _run(inputs, trace=False)
    return y

